# revision 6
# baseline (speedup 1.0000x reference)
"""BitNet GQA attention block on 8 TRN2 NeuronCores.

Three SPMD launches:
  L0: token-sharded prep — activation int8 quant (+transpose), |W| row sums,
      rope cos/sin tables (Cody-Waite range reduction + Sin LUT).
  L2: tensor-parallel (batch x 4 head-groups) — ternary weight quant of shards,
      exact-integer bf16 projections, rope (scales folded into tables),
      causal transposed-scores attention with hi/lo bf16 matmuls,
      natural-layout attention output with fused sumexp; Wo shard quant.
  L3: token-sharded O-projection (exact-integer bf16).

All quantized matmuls are exact: int8 activations / ternary weights are
integers representable in bf16, accumulated in fp32 PSUM.
"""
import sys
sys.path.insert(0, '/opt/trn_rl_repo')

import numpy as np
import ml_dtypes

import concourse.bass as bass
import concourse.bacc as bacc
import concourse.mybir as mybir
import concourse.tile as tile
from concourse import bass_utils

bf16 = ml_dtypes.bfloat16
F32 = mybir.dt.float32
BF16 = mybir.dt.bfloat16
I32 = mybir.dt.int32
AF = mybir.ActivationFunctionType
ALU = mybir.AluOpType
X = mybir.AxisListType.X

B, S, H = 2, 2048, 2048
NH, NKV, HD = 16, 4, 128
NCORES = 8
TOK = (B * S) // NCORES          # 512 tokens per core (L0/L3)
TS = S // NCORES                 # 256 table cols per core (L0)
HEADS_PC = NH // 4               # 4 q heads per L2 core
WROWS = (H + NKV * HD + NKV * HD + H) // NCORES   # 640 stacked weight rows

MAGIC = 1.5 * 2.0 ** 23          # fp32 rint trick
INV2PI = float(np.float32(1.0 / (2.0 * np.pi)))
C1 = 6.28125                     # 2*pi two-word split: C1 exact in 5 bits
C2 = float(np.float32(2.0 * np.pi - 6.28125))
HALF_PI = float(np.float32(np.pi / 2.0))
NEG_BIG = -1.0e9


def _rint(nc, pool, out_ap, in_ap, scale_ap=None, ternary=False):
    """out = rint(in * scale) half-to-even (matches jnp.round); optional
    clip to {-1,0,1} after rounding (ternary weights). Destroys in_ap."""
    t1 = in_ap
    if scale_ap is not None:
        nc.vector.tensor_scalar(t1, in_ap, scale_ap, MAGIC, ALU.mult, ALU.add)
    else:
        nc.vector.tensor_scalar(t1, in_ap, MAGIC, None, ALU.add)
    if ternary:
        # t1 is rint(w*ws)+MAGIC with integer ulp; these bounds round to
        # exactly MAGIC+1 / MAGIC-1 in fp32 -> clip(rint(.), -1, 1)
        nc.vector.tensor_scalar(t1, t1, MAGIC + 1.49, MAGIC - 1.49,
                                ALU.min, ALU.max)
    nc.vector.tensor_scalar(out_ap, t1, MAGIC, None, ALU.subtract)


# ----------------------------------------------------------------------------
# L0: prep
# ----------------------------------------------------------------------------
def build_l0():
    nc = bacc.Bacc("TRN2", target_bir_lowering=False, debug=False)
    xs = nc.dram_tensor("xs", [TOK, H], F32, kind="ExternalInput")
    wsl = nc.dram_tensor("wsl", [WROWS, H], F32, kind="ExternalInput")
    pos = nc.dram_tensor("pos", [1, TS], I32, kind="ExternalInput")
    invf = nc.dram_tensor("invf", [128, 1], F32, kind="ExternalInput")
    signc = nc.dram_tensor("signc", [128, 1], F32, kind="ExternalInput")

    xqT = nc.dram_tensor("xqT", [H, TOK], BF16, kind="ExternalOutput")
    s_inv = nc.dram_tensor("s_inv", [TOK], F32, kind="ExternalOutput")
    wabs = nc.dram_tensor("wabs", [WROWS, 16], F32, kind="ExternalOutput")
    cosT = nc.dram_tensor("cosT", [128, TS], F32, kind="ExternalOutput")
    sinTs = nc.dram_tensor("sinTs", [128, TS], F32, kind="ExternalOutput")

    with tile.TileContext(nc) as tc:
        with tc.tile_pool(name="sb", bufs=3) as sb, \
             tc.tile_pool(name="per", bufs=1) as per:
            # ---- hidden activation quant + transpose ----
            xqTall = per.tile([128, 16, TOK], BF16)
            for tt in range(TOK // 128):
                x_sb = sb.tile([128, H], F32, tag="x")
                nc.sync.dma_start(x_sb, xs.ap()[tt * 128:(tt + 1) * 128, :])
                mx = sb.tile([128, 1], F32, tag="mx")
                nc.vector.tensor_reduce(mx, x_sb, X, ALU.max,
                                        apply_absolute_value=True)
                nc.vector.tensor_scalar(mx, mx, 1e-5, None, ALU.max)
                rcp = sb.tile([128, 1], F32, tag="rcp")
                nc.vector.reciprocal(rcp, mx)
                s_col = sb.tile([128, 1], F32, tag="s")
                nc.vector.tensor_scalar(s_col, rcp, 127.0, None, ALU.mult)
                si_col = sb.tile([128, 1], F32, tag="si")
                nc.vector.tensor_scalar(si_col, mx, 1.0 / 127.0, None, ALU.mult)
                nc.sync.dma_start(s_inv.ap()[tt * 128:(tt + 1) * 128], si_col)
                xq = sb.tile([128, H], BF16, tag="xq")
                _rint(nc, sb, xq, x_sb, scale_ap=s_col)
                for hc in range(16):
                    nc.sync.dma_start_transpose(
                        xqTall[:, hc, tt * 128:(tt + 1) * 128],
                        xq[:, hc * 128:(hc + 1) * 128])
            nc.sync.dma_start(
                xqT.ap().rearrange("(c p) t -> p c t", p=128), xqTall)

            # ---- |W| row partial sums (16 chunks of 128, host combines) ----
            for wt in range(WROWS // 128):
                w_sb = sb.tile([128, H], F32, tag="x")
                nc.sync.dma_start(w_sb, wsl.ap()[wt * 128:(wt + 1) * 128, :])
                wa = sb.tile([128, 16], F32, tag="wa")
                nc.vector.tensor_reduce(
                    wa, w_sb.rearrange("p (c k) -> p c k", k=128), X, ALU.add,
                    apply_absolute_value=True)
                nc.sync.dma_start(
                    wabs.ap().rearrange("(i p) c -> p i c", p=128)[:, wt, :],
                    wa)

            # ---- rope tables ----
            pos_sb = sb.tile([1, TS], I32, tag="pos")
            nc.sync.dma_start(pos_sb, pos.ap())
            pb = sb.tile([128, TS], I32, tag="pb")
            nc.gpsimd.partition_broadcast(pb, pos_sb)
            posf = sb.tile([128, TS], F32, tag="posf")
            nc.vector.tensor_copy(posf, pb)
            invf_sb = sb.tile([128, 1], F32, tag="invf")
            nc.sync.dma_start(invf_sb, invf.ap())
            signc_sb = sb.tile([128, 1], F32, tag="signc")
            nc.sync.dma_start(signc_sb, signc.ap())
            ang = sb.tile([128, TS], F32, tag="ang")
            nc.vector.tensor_scalar(ang, posf, invf_sb, None, ALU.mult)

            def reduced(bias_quarter, tag):
                kf = sb.tile([128, TS], F32, tag=f"kf{tag}")
                if bias_quarter:
                    nc.vector.tensor_scalar(kf, ang, INV2PI, 0.25,
                                            ALU.mult, ALU.add)
                    nc.vector.tensor_scalar(kf, kf, MAGIC, MAGIC,
                                            ALU.add, ALU.subtract)
                else:
                    nc.vector.tensor_scalar(kf, ang, INV2PI, MAGIC,
                                            ALU.mult, ALU.add)
                    nc.vector.tensor_scalar(kf, kf, MAGIC, None, ALU.subtract)
                kc1 = sb.tile([128, TS], F32, tag=f"kc1{tag}")
                nc.vector.tensor_scalar(kc1, kf, C1, None, ALU.mult)
                r = sb.tile([128, TS], F32, tag=f"r{tag}")
                nc.vector.tensor_sub(r, ang, kc1)
                nc.vector.tensor_scalar(kc1, kf, C2, None, ALU.mult)
                nc.vector.tensor_sub(r, r, kc1)
                return r

            r_sin = reduced(False, "s")
            sin_sb = sb.tile([128, TS], F32, tag="sin")
            nc.scalar.activation(sin_sb, r_sin, AF.Sin)
            nc.vector.tensor_scalar(sin_sb, sin_sb, signc_sb, None, ALU.mult)
            nc.sync.dma_start(sinTs.ap(), sin_sb)

            r_cos = reduced(True, "c")
            nc.vector.tensor_scalar(r_cos, r_cos, HALF_PI, None, ALU.add)
            cos_sb = sb.tile([128, TS], F32, tag="cos")
            nc.scalar.activation(cos_sb, r_cos, AF.Sin)
            nc.sync.dma_start(cosT.ap(), cos_sb)

    nc.compile()
    return nc


# ----------------------------------------------------------------------------
# L2: projections + attention (one batch, 4 q heads, 1 kv head per core)
# ----------------------------------------------------------------------------
def build_l2():
    nc = bacc.Bacc("TRN2", target_bir_lowering=False, debug=False)
    xqT = nc.dram_tensor("xqT", [H, S], BF16, kind="ExternalInput")
    wq = nc.dram_tensor("wq", [512, H], F32, kind="ExternalInput")
    wk = nc.dram_tensor("wk", [128, H], F32, kind="ExternalInput")
    wv = nc.dram_tensor("wv", [128, H], F32, kind="ExternalInput")
    wo_sh = nc.dram_tensor("wo_sh", [256, H], F32, kind="ExternalInput")
    s_inv = nc.dram_tensor("s_inv", [1, S], F32, kind="ExternalInput")
    cosT = nc.dram_tensor("cosT", [128, S], F32, kind="ExternalInput")
    sinTs = nc.dram_tensor("sinTs", [128, S], F32, kind="ExternalInput")
    # consts: [ws_q, ws_k, ws_v, ws_o, qfac, vfac, ratio(k/q)]
    consts = nc.dram_tensor("consts", [1, 7], F32, kind="ExternalInput")

    av_sh = nc.dram_tensor("av_sh", [S, 512], F32, kind="ExternalOutput")
    woT_sh = nc.dram_tensor("woT_sh", [H, 256], BF16, kind="ExternalOutput")

    with tile.TileContext(nc) as tc:
        with tc.tile_pool(name="persist", bufs=1) as per, \
             tc.tile_pool(name="work", bufs=3) as wkp:
            c_sb = per.tile([1, 7], F32)
            nc.sync.dma_start(c_sb, consts.ap())
            cbc = per.tile([128, 7], F32)
            nc.gpsimd.partition_broadcast(cbc, c_sb)

            qr_hi = per.tile([128, HEADS_PC, S], BF16)
            qr_lo = per.tile([128, HEADS_PC, S], BF16)
            kr_hi = per.tile([128, S], BF16)
            kr_lo = per.tile([128, S], BF16)
            v_aug = per.tile([128, 16, 257], BF16)   # [vh | ones | vl]
            nc.vector.memset(v_aug[:, :, 128:129], 1.0)
            # per-token v scale = s_inv * vfac, [128, 16] columns
            sv = per.tile([128, 16], F32)
            nc.sync.dma_start(
                sv, s_inv.ap().rearrange("o (c p) -> (o p) c", p=128))
            nc.vector.tensor_scalar(sv, sv, cbc[:, 5:6], None, ALU.mult)

            with tc.tile_pool(name="projph", bufs=1) as pp, \
                 tc.tile_pool(name="xqs", bufs=2) as xqs, \
                 tc.tile_pool(name="ps_proj", bufs=3, space="PSUM") as psp:
                # ---- quantize weight shards (+ wo shard for L3) ----
                wqT = pp.tile([128, 16, 512], BF16)
                wkT = pp.tile([128, 16, 128], BF16)
                wvT = pp.tile([128, 16, 128], BF16)
                woTall = pp.tile([128, 16, 256], BF16)
                for (dram, rows, dst, wsi) in (
                        (wq, 512, wqT, 0), (wk, 128, wkT, 1),
                        (wv, 128, wvT, 2), (wo_sh, 256, woTall, 3)):
                    for rt in range(rows // 128):
                        w_sb = wkp.tile([128, H], F32, tag="w_in", bufs=2)
                        nc.sync.dma_start(
                            w_sb, dram.ap()[rt * 128:(rt + 1) * 128, :])
                        wq_sb = wkp.tile([128, H], BF16, tag="w_q", bufs=2)
                        _rint(nc, wkp, wq_sb, w_sb,
                              scale_ap=cbc[:, wsi:wsi + 1], ternary=True)
                        for hc in range(16):
                            nc.sync.dma_start_transpose(
                                dst[:, hc, rt * 128:(rt + 1) * 128],
                                wq_sb[:, hc * 128:(hc + 1) * 128])
                nc.sync.dma_start(
                    woT_sh.ap().rearrange("(c p) t -> p c t", p=128), woTall)

                # ---- rope tables with folded scales ----
                cos_q = pp.tile([128, S], F32)
                sin_q = pp.tile([128, S], F32)
                with tc.tile_pool(name="tbltmp", bufs=1) as tbl:
                    si_bc = tbl.tile([128, S], F32)
                    si_row = wkp.tile([1, S], F32, tag="si_row", bufs=1)
                    nc.sync.dma_start(si_row, s_inv.ap())
                    nc.gpsimd.partition_broadcast(si_bc, si_row)
                    nc.vector.tensor_scalar(si_bc, si_bc, cbc[:, 4:5], None,
                                            ALU.mult)      # s_inv * qfac
                    nc.sync.dma_start(cos_q, cosT.ap())
                    nc.vector.tensor_mul(cos_q, cos_q, si_bc)
                    nc.sync.dma_start(sin_q, sinTs.ap())
                    nc.vector.tensor_mul(sin_q, sin_q, si_bc)

                # ---- projections (streamed xqT column chunks) ----
                for tn in range(4):
                    tsl = slice(tn * 512, (tn + 1) * 512)
                    xq_t = xqs.tile([128, 16, 512], BF16, tag="xq")
                    nc.sync.dma_start(
                        xq_t, xqT.ap().rearrange(
                            "(c p) t -> p c t", p=128)[:, :, tsl])
                    for h in range(HEADS_PC + 1):   # 4 q heads then 1 k head
                        p_t = psp.tile([128, 512], F32, tag="pqk")
                        for hc in range(16):
                            lhsT = (wqT[:, hc, h * 128:(h + 1) * 128]
                                    if h < HEADS_PC else wkT[:, hc, :])
                            nc.tensor.matmul(p_t, lhsT, xq_t[:, hc, :],
                                             start=(hc == 0), stop=(hc == 15))
                        raw = wkp.tile([128, 512], F32, tag="qraw", bufs=2)
                        nc.scalar.activation(raw, p_t, AF.Copy)
                        rot = wkp.tile([128, 512], F32, tag="qrot", bufs=2)
                        nc.sync.dma_start(rot[0:64, :], raw[64:128, :])
                        nc.sync.dma_start(rot[64:128, :], raw[0:64, :])
                        m1 = wkp.tile([128, 512], F32, tag="m1", bufs=2)
                        nc.vector.tensor_mul(m1, raw, cos_q[:, tsl])
                        m2 = wkp.tile([128, 512], F32, tag="m2", bufs=2)
                        nc.vector.tensor_mul(m2, rot, sin_q[:, tsl])
                        nc.vector.tensor_add(m1, m1, m2)
                        if h == HEADS_PC:   # k head: cos_k = ratio * cos_q
                            nc.vector.tensor_scalar(m1, m1, cbc[:, 6:7],
                                                    None, ALU.mult)
                        hi = (qr_hi[:, h, tsl] if h < HEADS_PC
                              else kr_hi[:, tsl])
                        lo = (qr_lo[:, h, tsl] if h < HEADS_PC
                              else kr_lo[:, tsl])
                        nc.vector.tensor_copy(hi, m1)
                        nc.vector.tensor_sub(lo, m1, hi)
                    for kc in range(4 * tn, 4 * tn + 4):
                        p_v = psp.tile([128, 128], F32, tag="pv")
                        for hc in range(16):
                            nc.tensor.matmul(
                                p_v,
                                xq_t[:, hc, (kc - 4 * tn) * 128:
                                     (kc - 4 * tn + 1) * 128],
                                wvT[:, hc, :],
                                start=(hc == 0), stop=(hc == 15))
                        vf = wkp.tile([128, 128], F32, tag="vf")
                        nc.vector.tensor_scalar(vf, p_v, sv[:, kc:kc + 1],
                                                None, ALU.mult)
                        nc.vector.tensor_copy(v_aug[:, kc, 0:128], vf)
                        nc.vector.tensor_sub(v_aug[:, kc, 129:257], vf,
                                             v_aug[:, kc, 0:128])

            # ---- causal masks for diagonal tiles ----
            masks = per.tile([128, 4, 512], BF16)
            nc.vector.memset(masks, 0.0)
            for j in range(4):
                nc.gpsimd.affine_select(
                    masks[:, j, :], masks[:, j, :], pattern=[[1, 512]],
                    compare_op=ALU.is_ge, fill=NEG_BIG, base=-128 * j,
                    channel_multiplier=-1)

            # ---- attention ----
            with tc.tile_pool(name="attnout", bufs=1) as ao, \
                 tc.tile_pool(name="ps_sc", bufs=3, space="PSUM") as pssc, \
                 tc.tile_pool(name="ps_av", bufs=1, space="PSUM") as psav, \
                 tc.tile_pool(name="exps", bufs=4) as exps:
                attn = ao.tile([128, 16, 512], F32)
                for h in range(HEADS_PC):
                    for qc4 in range(4):
                        qsl = slice(qc4 * 512, (qc4 + 1) * 512)
                        nkt = 4 * qc4 + 4
                        avp = [psav.tile([128, 257], F32, tag=f"av{i}",
                                         name=f"avp{i}")
                               for i in range(4)]
                        for kt in range(nkt):
                            ksl = slice(kt * 128, (kt + 1) * 128)
                            sc_p = pssc.tile([128, 512], F32, tag="sc")
                            nc.tensor.matmul(sc_p, kr_hi[:, ksl],
                                             qr_hi[:, h, qsl],
                                             start=True, stop=False)
                            nc.tensor.matmul(sc_p, kr_hi[:, ksl],
                                             qr_lo[:, h, qsl],
                                             start=False, stop=False)
                            nc.tensor.matmul(sc_p, kr_lo[:, ksl],
                                             qr_hi[:, h, qsl],
                                             start=False, stop=True)
                            if kt >= 4 * qc4:
                                nc.vector.tensor_add(
                                    sc_p, sc_p, masks[:, kt - 4 * qc4, :])
                            p_hi = exps.tile([128, 512], BF16, tag="p_hi")
                            nc.scalar.activation(p_hi, sc_p, AF.Exp)
                            e_f = exps.tile([128, 512], F32, tag="e_f")
                            nc.scalar.activation(e_f, sc_p, AF.Exp)
                            p_lo = exps.tile([128, 512], BF16, tag="p_lo")
                            nc.vector.tensor_sub(p_lo, e_f, p_hi)
                            last = (kt == nkt - 1)
                            for qs in range(4):
                                qss = slice(qs * 128, (qs + 1) * 128)
                                nc.tensor.matmul(avp[qs], p_hi[:, qss],
                                                 v_aug[:, kt, :],
                                                 start=(kt == 0), stop=False)
                                nc.tensor.matmul(avp[qs][:, 0:129],
                                                 p_lo[:, qss],
                                                 v_aug[:, kt, 0:129],
                                                 start=False, stop=last)
                        for qs in range(4):
                            qc = qc4 * 4 + qs
                            se = wkp.tile([128, 1], F32, tag="se")
                            nc.vector.reciprocal(se, avp[qs][:, 128:129])
                            t0 = wkp.tile([128, 128], F32, tag="avt0")
                            nc.scalar.activation(t0, avp[qs][:, 0:128], AF.Copy)
                            t1 = wkp.tile([128, 128], F32, tag="avt1")
                            nc.vector.tensor_add(t1, t0,
                                                 avp[qs][:, 129:257])
                            nc.vector.tensor_scalar(
                                attn[:, qc, h * 128:(h + 1) * 128], t1, se,
                                None, ALU.mult)
                nc.sync.dma_start(
                    av_sh.ap().rearrange("(c p) d -> p c d", p=128), attn)

    nc.compile()
    return nc


# ----------------------------------------------------------------------------
# L3: O-projection, token-sharded
# ----------------------------------------------------------------------------
def build_l3():
    nc = bacc.Bacc("TRN2", target_bir_lowering=False, debug=False)
    av = nc.dram_tensor("av", [TOK, H], F32, kind="ExternalInput")
    woT = nc.dram_tensor("woT", [H, H], BF16, kind="ExternalInput")
    oc = nc.dram_tensor("oc", [1, 1], F32, kind="ExternalInput")  # 1/ws_o
    out = nc.dram_tensor("out", [TOK, H], F32, kind="ExternalOutput")

    with tile.TileContext(nc) as tc:
        with tc.tile_pool(name="persist", bufs=1) as per, \
             tc.tile_pool(name="work", bufs=3) as wkp, \
             tc.tile_pool(name="ps", bufs=4, space="PSUM") as ps:
            woT_sb = per.tile([128, 16, H], BF16)
            nc.sync.dma_start(
                woT_sb, woT.ap().rearrange("(c p) o -> p c o", p=128))
            oc_sb = per.tile([1, 1], F32)
            nc.sync.dma_start(oc_sb, oc.ap())
            oc_bc = per.tile([128, 1], F32)
            nc.gpsimd.partition_broadcast(oc_bc, oc_sb)
            for tt in range(TOK // 128):
                a_sb = wkp.tile([128, H], F32, tag="a")
                nc.sync.dma_start(a_sb, av.ap()[tt * 128:(tt + 1) * 128, :])
                mx = wkp.tile([128, 1], F32, tag="mx")
                nc.vector.tensor_reduce(mx, a_sb, X, ALU.max,
                                        apply_absolute_value=True)
                nc.vector.tensor_scalar(mx, mx, 1e-5, None, ALU.max)
                rcp = wkp.tile([128, 1], F32, tag="rcp")
                nc.vector.reciprocal(rcp, mx)
                s3 = wkp.tile([128, 1], F32, tag="s3")
                nc.vector.tensor_scalar(s3, rcp, 127.0, None, ALU.mult)
                si3 = wkp.tile([128, 1], F32, tag="si3")
                nc.vector.tensor_scalar(si3, mx, 1.0 / 127.0, None, ALU.mult)
                nc.vector.tensor_scalar(si3, si3, oc_bc, None, ALU.mult)
                aq = wkp.tile([128, H], BF16, tag="aq")
                _rint(nc, wkp, aq, a_sb, scale_ap=s3)
                aqT = wkp.tile([128, 16, 128], BF16, tag="aqT")
                for hc in range(16):
                    nc.sync.dma_start_transpose(
                        aqT[:, hc, :], aq[:, hc * 128:(hc + 1) * 128])
                for on in range(4):
                    osl = slice(on * 512, (on + 1) * 512)
                    p_t = ps.tile([128, 512], F32, tag="po")
                    for hc in range(16):
                        nc.tensor.matmul(p_t, aqT[:, hc, :],
                                         woT_sb[:, hc, osl],
                                         start=(hc == 0), stop=(hc == 15))
                    o_sb = wkp.tile([128, 512], F32, tag="o")
                    nc.vector.tensor_scalar(o_sb, p_t, si3, None, ALU.mult)
                    nc.sync.dma_start(
                        out.ap()[tt * 128:(tt + 1) * 128, osl], o_sb)
    nc.compile()
    return nc


# ----------------------------------------------------------------------------
# host orchestration
# ----------------------------------------------------------------------------
_CACHE = {}


def _get(name, builder):
    if name not in _CACHE:
        _CACHE[name] = builder()
    return _CACHE[name]


def kernel(hidden_states, attention_mask, Wq, Wk, Wv, Wo, position_ids):
    hidden_states = np.asarray(hidden_states, dtype=np.float32)
    Wq = np.asarray(Wq, dtype=np.float32)
    Wk = np.asarray(Wk, dtype=np.float32)
    Wv = np.asarray(Wv, dtype=np.float32)
    Wo = np.asarray(Wo, dtype=np.float32)
    position_ids = np.asarray(position_ids)
    if position_ids.dtype != np.int32:
        position_ids = position_ids.astype(np.int32)

    cores = list(range(NCORES))
    xflat = np.ascontiguousarray(hidden_states.reshape(B * S, H))
    wstack = np.concatenate([Wq, Wk, Wv, Wo], axis=0)  # [5120, H]

    inv64 = 1.0 / (10000.0 ** (np.arange(0, HD, 2, dtype=np.float64) / HD))
    invf = np.concatenate([inv64, inv64]).astype(np.float32)[:, None]
    signc = np.concatenate([-np.ones(64),
                            np.ones(64)]).astype(np.float32)[:, None]

    # ---------------- L0 ----------------
    nc0 = _get("l0", build_l0)
    in_maps = [dict(
        xs=np.ascontiguousarray(xflat[c * TOK:(c + 1) * TOK]),
        wsl=np.ascontiguousarray(wstack[c * WROWS:(c + 1) * WROWS]),
        pos=np.ascontiguousarray(position_ids[:, c * TS:(c + 1) * TS]),
        invf=invf, signc=signc) for c in cores]
    r0 = bass_utils.run_bass_kernel_spmd(nc0, in_maps, core_ids=cores).results

    wab = np.concatenate([r0[c]["wabs"] for c in cores], axis=0)  # [5120,16]
    sums = wab.astype(np.float64).sum(axis=1)
    bnds = [0, H, H + 512, H + 1024, 2 * H + 1024]
    means = [np.float32(sums[bnds[i]:bnds[i + 1]].sum()
                        / ((bnds[i + 1] - bnds[i]) * H)) for i in range(4)]
    ws_q, ws_k, ws_v, ws_o = [np.float32(1.0) / np.maximum(m, np.float32(1e-5))
                              for m in means]
    qfac = np.float32((1.0 / np.float64(ws_q)) / np.sqrt(np.float64(HD)))
    kfac = np.float32(1.0 / np.float64(ws_k))
    vfac = np.float32(1.0 / np.float64(ws_v))
    ofac = np.float32(1.0 / np.float64(ws_o))
    ratio = np.float32(np.float64(kfac) / np.float64(qfac))
    consts = np.array([[ws_q, ws_k, ws_v, ws_o, qfac, vfac, ratio]],
                      dtype=np.float32)

    cosT = np.concatenate([r0[c]["cosT"] for c in cores], axis=1)
    sinTs = np.concatenate([r0[c]["sinTs"] for c in cores], axis=1)
    s_inv_full = np.concatenate([r0[c]["s_inv"] for c in cores])  # [4096]
    xqT_b = [np.concatenate([r0[4 * b + i]["xqT"] for i in range(4)], axis=1)
             for b in range(B)]                                   # [H, S]

    # ---------------- L2 ----------------
    nc2 = _get("l2", build_l2)
    in_maps = []
    for c in cores:
        b, g = c // 4, c % 4
        in_maps.append(dict(
            xqT=xqT_b[b],
            wq=np.ascontiguousarray(Wq[g * 512:(g + 1) * 512]),
            wk=np.ascontiguousarray(Wk[g * 128:(g + 1) * 128]),
            wv=np.ascontiguousarray(Wv[g * 128:(g + 1) * 128]),
            wo_sh=np.ascontiguousarray(Wo[c * 256:(c + 1) * 256]),
            s_inv=np.ascontiguousarray(s_inv_full[b * S:(b + 1) * S])[None],
            cosT=cosT, sinTs=sinTs, consts=consts))
    r2 = bass_utils.run_bass_kernel_spmd(nc2, in_maps, core_ids=cores).results

    av_full = np.empty((B * S, H), dtype=np.float32)
    for c in cores:
        b, g = c // 4, c % 4
        av_full[b * S:(b + 1) * S, g * 512:(g + 1) * 512] = r2[c]["av_sh"]
    woT = np.concatenate([r2[c]["woT_sh"] for c in cores], axis=1)  # [H, H]

    # ---------------- L3 ----------------
    nc3 = _get("l3", build_l3)
    oc = np.array([[ofac]], dtype=np.float32)
    in_maps = [dict(av=np.ascontiguousarray(av_full[c * TOK:(c + 1) * TOK]),
                    woT=woT, oc=oc) for c in cores]
    r3 = bass_utils.run_bass_kernel_spmd(nc3, in_maps, core_ids=cores).results

    out = np.concatenate([r3[c]["out"] for c in cores], axis=0)
    return out.reshape(B, S, H)
